# revision 38
# baseline (speedup 1.0000x reference)
"""Trainium2 Bass kernel for a binarized-conv BasicBlock (dense_cnn).

Computation (matches the reference nn.Module):
    out = clip(BN2(conv3x3(binarize(clip(BN1(conv3x3(binarize(x), binarize(w1))))),
                  binarize(w2)) + x))
with training-mode (batch-stats) BN over the full 64-image batch.

Strategy:
  - Data-parallel over batch: 8 images per core on 8 NeuronCores.
  - x/w1/w2 are host-cast to bf16 (signs exact; the bf16 residual costs
    ~3e-4 rel err vs the 2e-2 gate) halving input DMA to ~5.5 MB/core.
  - Binarized 3x3 conv as 9 accumulating DoubleRow fp8 PE matmuls (K=256)
    per [128, 392] output tile over zero-padded [128, 2, 30, 32] fp8
    activation tiles; the +-1 / +-0.5 encodings are exact in fp8 and the
    per-block scale is folded into the eviction scale; PSUM is fp32.
  - Weight prep: one Sign to bf16, then 1-cycle/row PE transposes (bf16
    identity), then PSUM->fp8 copies split Scalar/Vector. w2's Sign runs
    on Vector as is_ge-0.5 (+-0.5 weights) to keep Scalar free.
  - BN1 + hardtanh + binarize collapses to a per-channel threshold
    compare; sync-BN via a [128, 4] AllReduce whose pre-path (GpSimd
    add-tree reduce) and post-path (GpSimd threshold chain) avoid the
    eviction-critical queues. Imgs 0-1 binarize on Scalar (Sign with the
    threshold as bias; +-1 values, eviction scale compensated) for both
    convs so neither conv's start waits on the Vector queue; w2
    transposes bridge the BN1 sync wait on the PE.
  - conv2 runs output-block-major so BN2 splits per block: AR2a runs
    under conv2-ob1's PE time; its readback-dependent param chain and all
    output tiles (affine + clamp + one 400 KB DMA each) are emitted after
    every eviction so an in-order schedule can never stall the PE. The
    ob0 output stream overlaps AR2b; only ob1's burst is tail-exposed.
  - Padded activation buffers get border-only memsets and are recycled
    between conv1 and conv2 inputs (interior is always fully rewritten);
    a warmup collective absorbs the ncfw wake before the first BN sync.
"""

import os
import sys

import numpy as np


def _ensure_paths():
    for p in ("/opt/trn_rl_repo", "/root/.axon_site/_ro/trn_rl_repo"):
        if p not in sys.path and os.path.isdir(p):
            sys.path.append(p)


try:
    from concourse import bacc, mybir, tile  # noqa: F401
except ImportError:
    _ensure_paths()
    from concourse import bacc, mybir, tile  # noqa: F401

from concourse.bass_utils import run_bass_kernel_spmd
from concourse.masks import make_identity

N_CORES = 8
IMGS = 8          # images per core (64 / 8)
C = 256
CB = 2            # channel blocks of 128
H = W = 28
HP = WP = 30      # zero-padded spatial
PIX = H * W       # 784
HALF = PIX // 2   # 392 (one PSUM bank of fp32)
NT = 64 * PIX     # BN count over the GLOBAL batch (N*H*W)
EPS = 1e-5
NPAD = IMGS + 2   # physical padded-activation buffers (recycled)

F32 = mybir.dt.float32
BF16 = mybir.dt.bfloat16
FP8 = mybir.dt.float8e4
AF = mybir.ActivationFunctionType
ALU = mybir.AluOpType
DR = mybir.MatmulPerfMode.DoubleRow

# padded fp8 activation layout: [128, 2 kblocks, 30 rows, 32 cols]
RP = 32           # row pitch (28 cols + pad, %16 bytes)
KP = HP * RP      # per-kblock pitch = 960

_PROGRAM = None


def _build_program():
    nc = bacc.Bacc("TRN2", target_bir_lowering=False, debug=False,
                   num_devices=N_CORES)

    x_in = nc.dram_tensor("x", [IMGS, C, H, W], BF16, kind="ExternalInput").ap()
    w1_in = nc.dram_tensor("w1", [C, C, 3, 3], BF16, kind="ExternalInput").ap()
    w2_in = nc.dram_tensor("w2", [C, C, 3, 3], BF16, kind="ExternalInput").ap()
    g1_in = nc.dram_tensor("gamma1", [C], F32, kind="ExternalInput").ap()
    b1_in = nc.dram_tensor("beta1", [C], F32, kind="ExternalInput").ap()
    g2_in = nc.dram_tensor("gamma2", [C], F32, kind="ExternalInput").ap()
    b2_in = nc.dram_tensor("beta2", [C], F32, kind="ExternalInput").ap()
    out_d = nc.dram_tensor("out", [IMGS, C, H, W], F32, kind="ExternalOutput").ap()

    groups = [list(range(N_CORES))]

    with tile.TileContext(nc) as tc:
        with (
            tc.tile_pool(name="consts", bufs=1) as p_const,
            tc.tile_pool(name="wstage", bufs=3) as p_wstage,
            tc.tile_pool(name="wsign", bufs=2) as p_wsign,
            tc.tile_pool(name="wt", bufs=2 * 9 * 2) as p_wt,
            tc.tile_pool(name="xp", bufs=IMGS * CB) as p_x,
            tc.tile_pool(name="apad", bufs=NPAD) as p_apad,
            tc.tile_pool(name="yz", bufs=IMGS * CB) as p_yz,
            tc.tile_pool(name="sq", bufs=2) as p_sq,
            tc.tile_pool(name="o1", bufs=8) as p_o1,
            tc.tile_pool(name="ps", bufs=8, space="PSUM") as p_ps,
            tc.tile_pool(name="dram", bufs=1, space="DRAM") as p_dram,
        ):

            # bf16 identity: weight transposes run at 1 cycle/row in bf16
            # (vs 2 for f32), halving their PE cost.
            ident = p_const.tile([128, 128], BF16, name="ident")
            make_identity(nc, ident)

            def dma_chunked(out_ap, in_ap, parts, engs):
                """Split a big DMA along the last free dim so the transfer
                spreads across several DMA engines; rotate the issuing
                queue over `engs` so the ~0.6us per-issue cost is
                parallelized across sequencers."""
                n = out_ap.shape[-1]
                step = (n + parts - 1) // parts
                for ci, a in enumerate(range(0, n, step)):
                    b = min(a + step, n)
                    engs[ci % len(engs)].dma_start(
                        out=out_ap[:, a:b], in_=in_ap[:, a:b])

            # gamma/beta as [128, 2] (col = channel block)
            def load_cvec(src, nm):
                t = p_const.tile([128, CB], F32, name=nm)
                nc.sync.dma_start(out=t, in_=src.rearrange("(b p) -> p b", p=128))
                return t

            # gamma/beta loads deferred (cvec_loads) so they don't occupy
            # DMA slots during the critical head window.
            cv = {}

            def cvec_loads():
                g1t = load_cvec(g1_in, "g1t")
                b1t = load_cvec(b1_in, "b1t")
                rg1 = p_const.tile([128, CB], F32, name="rg1")
                nc.vector.reciprocal(rg1, g1t)
                bg1 = p_const.tile([128, CB], F32, name="bg1")
                nc.vector.tensor_mul(bg1, b1t, rg1)
                cv["bg1"] = bg1
                cv["g2t"] = load_cvec(g2_in, "g2t")
                cv["b2t"] = load_cvec(b2_in, "b2t")

            # per-channel stat accumulators, one column per (img, half)
            def stat_tiles(nm):
                return [p_const.tile([128, IMGS * 2], F32, name=f"{nm}{ob}")
                        for ob in range(CB)]

            st1s, st1q = stat_tiles("st1s"), stat_tiles("st1q")
            st2s, st2q = stat_tiles("st2s"), stat_tiles("st2q")

            # ---- padded fp8 activation buffers: border-only memsets ----
            # The binarize writes always cover [1:29, 1:29]; only the halo
            # (row 0, row 29, col 0, cols 29-31) must be zero, and it stays
            # zero when a buffer is recycled between conv1 and conv2 inputs.
            pad = [p_apad.tile([128, CB * KP], FP8, tag="apad", name=f"pad{i}")
                   for i in range(NPAD)]

            def memset_border(t):
                a4 = t.rearrange("p (k r c) -> p k r c", k=CB, r=HP)
                for b in range(CB):
                    nc.gpsimd.memset(a4[:, b, 0, :], 0.0)
                    nc.gpsimd.memset(a4[:, b, 29, :], 0.0)
                    nc.gpsimd.memset(a4[:, b, 1:29, 0:1], 0.0)
                    nc.gpsimd.memset(a4[:, b, 1:29, 29:32], 0.0)

            xsign = [pad[n] for n in range(IMGS)]
            b2a = [pad[(IMGS + n) % NPAD] for n in range(IMGS)]

            # ---- weight staging (DMA) and prep: one big Sign to bf16, then
            # cheap 1-cycle/row PE transposes, then PSUM->fp8 copies ----
            def stage_w(w_in, wi, ob):
                wst = p_wstage.tile([128, C * 9], BF16, tag="wst",
                                    name=f"wst{wi}_{ob}")
                dma_chunked(
                    wst,
                    w_in[ob * 128:(ob + 1) * 128].rearrange(
                        "o i ky kx -> o (i ky kx)"),
                    parts=6,
                    engs=[nc.sync, nc.scalar] if (wi, ob) == (1, 0)
                    else [nc.sync])
                return wst

            def prep_w(wst, wi, ob, wt, on_vector=False):
                """Sign + transpose + PSUM->fp8 copy for one weight block.
                on_vector: run the Sign on the Vector engine as
                is_ge - 0.5 (weights become +-0.5 instead of +-1; the
                eviction scale compensates), keeping the Scalar queue free.
                The PSUM copies alternate Scalar/Vector."""
                wsg = p_wsign.tile([128, C * 9], BF16, tag="wsg",
                                   name=f"wsg{wi}_{ob}")
                if on_vector:
                    nc.vector.tensor_scalar(out=wsg, in0=wst, scalar1=0.0,
                                            scalar2=0.5, op0=ALU.is_ge,
                                            op1=ALU.subtract)
                else:
                    nc.scalar.activation(wsg, wst, AF.Sign)
                w3 = wsg.rearrange("p (i t) -> p i t", t=9)
                for tap in range(9):
                    t = p_wt.tile([128, CB * 128], FP8, tag="wt",
                                  name=f"wt{wi}_{tap}_{ob}")
                    wt[(tap, ob)] = t
                    for kb in range(CB):
                        ps = p_ps.tile([128, 128], BF16, tag="ps",
                                       name=f"pst{wi}_{ob}_{kb}_{tap}")
                        nc.tensor.transpose(
                            ps, w3[:, kb * 128:(kb + 1) * 128, tap], ident)
                        if kb == 0:
                            nc.scalar.activation(
                                t[:, kb * 128:(kb + 1) * 128], ps, AF.Copy)
                        else:
                            nc.vector.tensor_scalar(
                                out=t[:, kb * 128:(kb + 1) * 128], in0=ps,
                                scalar1=1.0, scalar2=None, op0=ALU.mult)

            # ---- x: load raw f32 (kept for the residual), sign -> fp8 ----
            xt = [[None] * CB for _ in range(IMGS)]

            def load_x(n, parts):
                a4 = xsign[n].rearrange("p (k r c) -> p k r c", k=CB, r=HP)
                for b in range(CB):
                    xr = p_x.tile([128, PIX], BF16, tag="xp", name=f"x_{n}_{b}")
                    dma_chunked(
                        xr,
                        x_in[n, b * 128:(b + 1) * 128].rearrange(
                            "c h w -> c (h w)"),
                        parts=parts,
                        engs=[nc.sync, nc.scalar] if n < 2 else [nc.sync])
                    xt[n][b] = xr
                    if n < 2:
                        # Scalar-engine Sign (+-1 acts, evict1 scale 1.0):
                        # keeps the head's Vector queue free for the
                        # weight-prep PSUM copies.
                        nc.scalar.activation(
                            a4[:, b, 1:29, 1:29],
                            xr.rearrange("p (h w) -> p h w", h=H),
                            AF.Sign)
                    else:
                        nc.vector.tensor_scalar(
                            out=a4[:, b, 1:29, 1:29],
                            in0=xr.rearrange("p (h w) -> p h w", h=H),
                            scalar1=0.0, scalar2=0.5,
                            op0=ALU.is_ge, op1=ALU.subtract)

            # head: w1-ob0 staging + the first image pair lead the DMA
            # queues (nothing else competes until they are in flight), then
            # w1-ob1 and the x stream; w2 staging is deferred until after
            # conv1 is emitted so its transfers ride behind the x stream.
            # GpSimd only runs the halo memsets here; the first two pad
            # buffers are zeroed before the img0/img1 binarize needs them.
            wt1, wt2 = {}, {}
            memset_border(pad[0])
            memset_border(pad[1])
            # Warmup collective: absorbs the ~11us ncfw wake + first-mesh-op
            # overhead on stream 0 while conv1 runs, so the BN sync
            # AllReduces behave like warm ops. Emitted after the first two
            # halo memsets: its trigger blocks the GpSimd queue ~7us.
            ccw_i = p_dram.tile([128, 1], F32, name="ccw_i")
            ccw_o = p_dram.tile([128, 1], F32, name="ccw_o")
            zz = p_const.tile([128, 1], F32, name="zz")
            nc.vector.memset(zz, 0.0)
            nc.scalar.dma_start(out=ccw_i, in_=zz)
            nc.gpsimd.collective_compute(
                "AllReduce", ALU.add, replica_groups=groups,
                ins=[ccw_i.opt()], outs=[ccw_o.opt()])
            load_x(0, parts=1)
            ws10 = stage_w(w1_in, 1, 0)
            load_x(1, parts=1)
            prep_w(ws10, 1, 0, wt1)
            ws11 = stage_w(w1_in, 1, 1)
            cvec_loads()
            for n in range(2, IMGS):
                load_x(n, parts=2 if n < 4 else 1)
            for i in range(2, NPAD):
                memset_border(pad[i])

            # ---- conv: 9 DoubleRow matmuls (K=256) per [128, 392] PSUM ----
            def emit_group(wt, act, evict, pair, ob, tiles=None):
                if tiles is None:
                    tiles = [(n, half)
                             for n in (2 * pair, 2 * pair + 1)
                             for half in range(2)]
                pss = {}
                for (n, half) in tiles:
                    pss[(n, half)] = p_ps.tile(
                        [128, HALF], F32, tag="ps",
                        name=f"ps_{ob}_{n}_{half}")
                for tap in range(9):
                    dy, dx = divmod(tap, 3)
                    w3 = wt[(tap, ob)].rearrange(
                        "p (k o) -> p k o", k=CB)
                    for (n, half) in tiles:
                        a4 = act[n].rearrange(
                            "p (k r c) -> p k r c", k=CB, r=HP)
                        rhs = a4[:, :, dy + half * 14: dy + half * 14 + 14,
                                 dx: dx + W]
                        nc.tensor.matmul(pss[(n, half)], w3, rhs,
                                         start=(tap == 0),
                                         stop=(tap == 8),
                                         perf_mode=DR)
                for (n, half) in tiles:
                    evict(n, ob, half, pss[(n, half)])

            # ---- conv1 eviction: copy PSUM->y1 with sum, square w/ sumsq ----
            y1 = [[None] * CB for _ in range(IMGS)]

            def evict1(n, ob, half, ps):
                if y1[n][ob] is None:
                    y1[n][ob] = p_yz.tile([128, PIX], F32, tag="yz",
                                          name=f"y1_{n}_{ob}")
                idx = n * 2 + half
                ysl = y1[n][ob][:, half * HALF:(half + 1) * HALF]
                nc.scalar.activation(ysl, ps, AF.Copy,
                                     scale=1.0 if n < 2 else 2.0,
                                     accum_out=st1s[ob][:, idx:idx + 1])
                sq = p_sq.tile([128, HALF], F32, tag="sq")
                nc.vector.scalar_tensor_tensor(
                    out=sq, in0=ysl, scalar=1.0, in1=ysl,
                    op0=ALU.mult, op1=ALU.mult,
                    accum_out=st1q[ob][:, idx:idx + 1])

            # conv1: image-major (pairs outer) relaxes the x-load deadlines.
            # The first two groups are single-image so the very first
            # matmuls need only img0; w1-ob1 prep slots in between.
            emit_group(wt1, xsign, evict1, 0, 0, tiles=[(0, 0), (0, 1)])
            emit_group(wt1, xsign, evict1, 0, 0, tiles=[(1, 0), (1, 1)])
            prep_w(ws11, 1, 1, wt1)
            emit_group(wt1, xsign, evict1, 0, 1)
            for pair in range(1, IMGS // 2):
                for ob in range(CB):
                    emit_group(wt1, xsign, evict1, pair, ob)

            # w2 staging: issued here so the transfers queue up behind the
            # x stream and land well before the BN1 sync window.
            ws20 = stage_w(w2_in, 2, 0)
            ws21 = stage_w(w2_in, 2, 1)

            # ---- BN1: AllReduce global sums, derive per-channel thresholds ----
            # free-axis reduce as a GpSimd add-tree: the GpSimd queue
            # holds the whole sync path (reduce -> cc-in DMA -> trigger ->
            # readback -> param chain), so busy Vector/Scalar queues can
            # never delay a BN sync.
            rs8 = p_const.tile([128, 8], F32, name="rs8")
            rs4 = p_const.tile([128, 4], F32, name="rs4")
            rs2 = p_const.tile([128, 2], F32, name="rs2")

            def tree_reduce(dst, src_t):
                nc.gpsimd.tensor_tensor(out=rs8, in0=src_t[:, 0:8],
                                        in1=src_t[:, 8:16], op=ALU.add)
                nc.gpsimd.tensor_tensor(out=rs4, in0=rs8[:, 0:4],
                                        in1=rs8[:, 4:8], op=ALU.add)
                nc.gpsimd.tensor_tensor(out=rs2, in0=rs4[:, 0:2],
                                        in1=rs4[:, 2:4], op=ALU.add)
                nc.gpsimd.tensor_tensor(out=dst, in0=rs2[:, 0:1],
                                        in1=rs2[:, 1:2], op=ALU.add)

            pk1 = p_const.tile([128, 2 * CB], F32, name="pk1")
            for ob in range(CB):
                tree_reduce(pk1[:, 2 * ob:2 * ob + 1], st1s[ob])
                tree_reduce(pk1[:, 2 * ob + 1:2 * ob + 2], st1q[ob])
            cc1i = p_dram.tile([128, 2 * CB], F32, name="cc1i")
            cc1o = p_dram.tile([128, 2 * CB], F32, name="cc1o")
            nc.sync.dma_start(out=cc1i, in_=pk1)
            nc.gpsimd.collective_compute(
                "AllReduce", ALU.add, replica_groups=groups,
                ins=[cc1i.opt()], outs=[cc1o.opt()])
            red1 = p_const.tile([128, 2 * CB], F32, name="red1")
            nc.sync.dma_start(out=red1, in_=cc1o)
            r3 = red1.rearrange("p (b k) -> p b k", k=2)

            # w2 prep fills the sync-BN wait on the PE
            prep_w(ws20, 2, 0, wt2, on_vector=True)
            prep_w(ws21, 2, 1, wt2, on_vector=True)

            # threshold chain on GpSimd (idle here) to keep it off the
            # busier Vector queue; only the Sqrt hops to Scalar.
            m1 = p_const.tile([128, CB], F32, name="m1")
            nc.gpsimd.tensor_scalar(out=m1, in0=r3[:, :, 0], scalar1=1.0 / NT,
                                    scalar2=None, op0=ALU.mult)
            mm1 = p_const.tile([128, CB], F32, name="mm1")
            nc.gpsimd.tensor_mul(mm1, m1, m1)
            v1e = p_const.tile([128, CB], F32, name="v1e")
            nc.gpsimd.tensor_scalar(out=v1e, in0=r3[:, :, 1],
                                    scalar1=1.0 / NT, scalar2=EPS,
                                    op0=ALU.mult, op1=ALU.add)
            v1 = p_const.tile([128, CB], F32, name="v1")
            nc.gpsimd.tensor_sub(v1, v1e, mm1)
            sd1 = p_const.tile([128, CB], F32, name="sd1")
            nc.scalar.activation(sd1, v1, AF.Sqrt)
            tb1 = p_const.tile([128, CB], F32, name="tb1")
            nc.gpsimd.tensor_mul(tb1, cv["bg1"], sd1)
            thr1 = p_const.tile([128, CB], F32, name="thr1")
            nc.gpsimd.tensor_sub(thr1, m1, tb1)
            # negated threshold: bias for the Scalar-engine Sign binarize
            thn1 = p_const.tile([128, CB], F32, name="thn1")
            nc.gpsimd.tensor_sub(thn1, tb1, m1)

            # ---- binarize(BN1(y1)): imgs 0-1 on Scalar as Sign(y - thr)
            # (values +-1, conv2 evict scale 1.0) so conv2 starts sooner;
            # the rest on Vector as is_ge - 0.5 (values +-0.5, scale 2). ----
            for n in range(IMGS):
                a4 = b2a[n].rearrange("p (k r c) -> p k r c", k=CB, r=HP)
                for b in range(CB):
                    if n < 2:
                        nc.scalar.activation(
                            a4[:, b, 1:29, 1:29],
                            y1[n][b].rearrange("p (h w) -> p h w", h=H),
                            AF.Sign, bias=thn1[:, b:b + 1])
                    else:
                        nc.vector.tensor_scalar(
                            out=a4[:, b, 1:29, 1:29],
                            in0=y1[n][b].rearrange("p (h w) -> p h w", h=H),
                            scalar1=thr1[:, b:b + 1], scalar2=0.5,
                            op0=ALU.is_ge, op1=ALU.subtract)

            # ---- conv2 eviction: z = s*psum + x (fused sum), square ----
            z = [[None] * CB for _ in range(IMGS)]

            def evict2(n, ob, half, ps):
                if z[n][ob] is None:
                    z[n][ob] = p_yz.tile([128, PIX], F32, tag="yz",
                                         name=f"z_{n}_{ob}")
                idx = n * 2 + half
                zsl = z[n][ob][:, half * HALF:(half + 1) * HALF]
                nc.vector.scalar_tensor_tensor(
                    out=zsl, in0=ps, scalar=2.0 if n < 2 else 4.0,
                    in1=xt[n][ob][:, half * HALF:(half + 1) * HALF],
                    op0=ALU.mult, op1=ALU.add,
                    accum_out=st2s[ob][:, idx:idx + 1])
                sq = p_sq.tile([128, HALF], F32, tag="sq")
                if half == 0 and n < 6:
                    nc.scalar.activation(sq, zsl, AF.Square,
                                         accum_out=st2q[ob][:, idx:idx + 1])
                else:
                    nc.vector.scalar_tensor_tensor(
                        out=sq, in0=zsl, scalar=1.0, in1=zsl,
                        op0=ALU.mult, op1=ALU.mult,
                        accum_out=st2q[ob][:, idx:idx + 1])

            # ---- BN2 per output block: reduce, AllReduce, affine params.
            # The post-AR chain runs on GpSimd (idle then) + one Scalar
            # Rsqrt so the busy Vector/Sync queues never gate fs/fb. ----
            def bn2_ar(ob):
                pk = p_const.tile([128, 2], F32, name=f"pk2_{ob}")
                tree_reduce(pk[:, 0:1], st2s[ob])
                tree_reduce(pk[:, 1:2], st2q[ob])
                cci = p_dram.tile([128, 2], F32, name=f"cc2i_{ob}")
                cco = p_dram.tile([128, 2], F32, name=f"cc2o_{ob}")
                nc.sync.dma_start(out=cci, in_=pk)
                nc.gpsimd.collective_compute(
                    "AllReduce", ALU.add, replica_groups=groups,
                    ins=[cci.opt()], outs=[cco.opt()])
                red = p_const.tile([128, 2], F32, name=f"red2_{ob}")
                nc.sync.dma_start(out=red, in_=cco)
                return red

            def bn2_chain(red, ob):
                # post-AR chain on Vector, emitted only after every conv2
                # eviction so a strict in-order schedule cannot stall them
                m2 = p_const.tile([128, 1], F32, name=f"m2_{ob}")
                nc.vector.tensor_scalar(out=m2, in0=red[:, 0:1],
                                        scalar1=1.0 / NT, scalar2=None,
                                        op0=ALU.mult)
                mm2 = p_const.tile([128, 1], F32, name=f"mm2_{ob}")
                nc.vector.tensor_scalar(out=mm2, in0=red[:, 0:1],
                                        scalar1=red[:, 0:1],
                                        scalar2=1.0 / (NT * NT),
                                        op0=ALU.mult, op1=ALU.mult)
                v2f = p_const.tile([128, 1], F32, name=f"v2f_{ob}")
                nc.vector.tensor_scalar(out=v2f, in0=red[:, 1:2],
                                        scalar1=1.0 / NT, scalar2=EPS,
                                        op0=ALU.mult, op1=ALU.add)
                v2 = p_const.tile([128, 1], F32, name=f"v2_{ob}")
                nc.vector.tensor_sub(v2, v2f, mm2)
                rc2 = p_const.tile([128, 1], F32, name=f"rc2_{ob}")
                nc.vector.reciprocal(rc2, v2)
                rstd = p_const.tile([128, 1], F32, name=f"rstd_{ob}")
                nc.scalar.activation(rstd, rc2, AF.Sqrt)
                fs = p_const.tile([128, 1], F32, name=f"fs_{ob}")
                nc.vector.tensor_mul(fs, cv["g2t"][:, ob:ob + 1], rstd)
                msc = p_const.tile([128, 1], F32, name=f"msc_{ob}")
                nc.vector.tensor_mul(msc, m2, fs)
                fb = p_const.tile([128, 1], F32, name=f"fb_{ob}")
                nc.vector.tensor_sub(fb, cv["b2t"][:, ob:ob + 1], msc)
                return fs, fb

            # ---- final: clip(z * fscale + fbias) -> DRAM ----
            def out_tile(n, ob, fs, fb, aff_eng, clamp_eng, dma_engs):
                o1 = p_o1.tile([128, PIX], F32, tag="o1")
                if aff_eng is nc.scalar:
                    nc.scalar.activation(o1, z[n][ob], AF.Identity,
                                         bias=fb, scale=fs)
                else:
                    aff_eng.tensor_scalar(
                        out=o1, in0=z[n][ob], scalar1=fs, scalar2=fb,
                        op0=ALU.mult, op1=ALU.add)
                clamp_eng.tensor_scalar(out=o1, in0=o1, scalar1=-1.0,
                                        scalar2=1.0, op0=ALU.max, op1=ALU.min)
                dma_chunked(
                    out_d[n, ob * 128:(ob + 1) * 128].rearrange(
                        "c h w -> c (h w)"),
                    o1, parts=1, engs=dma_engs)

            # conv2-ob0, then its BN sync; the ob0 output phase rides on
            # GpSimd (affine) + Vector (clamp; GpSimd MAX/MIN is ~10x slow)
            # + the sync DMA queue underneath conv2-ob1's PE time.
            for pair in range(IMGS // 2):
                emit_group(wt2, b2a, evict2, pair, 0)
            red2a = bn2_ar(0)
            for pair in range(IMGS // 2):
                emit_group(wt2, b2a, evict2, pair, 1)
            red2b = bn2_ar(1)

            fs0, fb0 = bn2_chain(red2a, 0)
            for n in range(IMGS):
                aff = nc.scalar if n % 2 == 0 else nc.vector
                out_tile(n, 0, fs0, fb0, aff, nc.vector, [nc.sync])

            fs1, fb1 = bn2_chain(red2b, 1)
            for n in range(IMGS):
                aff = nc.scalar if n % 2 == 0 else nc.gpsimd
                dmae = [[nc.sync], [nc.scalar]][n % 2]
                out_tile(n, 1, fs1, fb1, aff, nc.vector, dmae)

    nc.compile()
    return nc


def _get_program():
    global _PROGRAM
    if _PROGRAM is None:
        _PROGRAM = _build_program()
    return _PROGRAM


def run_sharded(inputs, **spmd_kwargs):
    """Shard inputs across 8 cores, run, and gather. Returns (out, results)."""
    nc = _get_program()
    bf16 = mybir.dt.np(mybir.dt.bfloat16)
    x = np.ascontiguousarray(np.asarray(inputs["x"]).astype(bf16))
    base = {}
    for k in ("w1", "w2"):
        base[k] = np.ascontiguousarray(np.asarray(inputs[k]).astype(bf16))
    for k in ("gamma1", "beta1", "gamma2", "beta2"):
        base[k] = np.ascontiguousarray(
            np.asarray(inputs[k], dtype=np.float32))
    shards = np.split(x, N_CORES, axis=0)
    in_maps = [{"x": shards[i], **base} for i in range(N_CORES)]
    res = run_bass_kernel_spmd(nc, in_maps, core_ids=list(range(N_CORES)),
                               **spmd_kwargs)
    out = np.concatenate([res.results[i]["out"] for i in range(N_CORES)],
                         axis=0).astype(np.float32)
    return out, res


def kernel(**inputs):
    out, _ = run_sharded(inputs)
    return out


# revision 41
# speedup vs baseline: 1.0397x; 1.0397x over previous
"""Trainium2 Bass kernel for a binarized-conv BasicBlock (dense_cnn).

Computation (matches the reference nn.Module):
    out = clip(BN2(conv3x3(binarize(clip(BN1(conv3x3(binarize(x), binarize(w1))))),
                  binarize(w2)) + x))
with training-mode (batch-stats) BN over the full 64-image batch.

Strategy:
  - Data-parallel over batch: 8 images per core on 8 NeuronCores.
  - x/w1/w2 are host-cast to bf16 (signs exact; the bf16 residual costs
    ~3e-4 rel err vs the 2e-2 gate) halving input DMA to ~5.5 MB/core.
  - Binarized 3x3 conv as 9 accumulating DoubleRow fp8 PE matmuls (K=256)
    per [128, 392] output tile over zero-padded [128, 2, 30, 32] fp8
    activation tiles; the +-1 / +-0.5 encodings are exact in fp8 and the
    per-block scale is folded into the eviction scale; PSUM is fp32.
  - Weight prep: one Sign to bf16, then 1-cycle/row PE transposes (bf16
    identity), then PSUM->fp8 copies split Scalar/Vector. w2's Sign runs
    on Vector as is_ge-0.5 (+-0.5 weights) to keep Scalar free.
  - BN1 + hardtanh + binarize collapses to a per-channel threshold
    compare; sync-BN via a [128, 4] AllReduce whose pre-path (GpSimd
    add-tree reduce) and post-path (GpSimd threshold chain) avoid the
    eviction-critical queues. Imgs 0-1 binarize on Scalar (Sign with the
    threshold as bias; +-1 values, eviction scale compensated) for both
    convs so neither conv's start waits on the Vector queue; w2
    transposes bridge the BN1 sync wait on the PE.
  - conv2 runs output-block-major so BN2 splits per block: AR2a runs
    under conv2-ob1's PE time; its readback-dependent param chain and all
    output tiles (affine + clamp + one 400 KB DMA each) are emitted after
    every eviction so an in-order schedule can never stall the PE. The
    ob0 output stream overlaps AR2b; only ob1's burst is tail-exposed.
  - Padded activation buffers get border-only memsets and are recycled
    between conv1 and conv2 inputs (interior is always fully rewritten);
    a warmup collective absorbs the ncfw wake before the first BN sync.
"""

import os
import sys

import numpy as np


def _ensure_paths():
    for p in ("/opt/trn_rl_repo", "/root/.axon_site/_ro/trn_rl_repo"):
        if p not in sys.path and os.path.isdir(p):
            sys.path.append(p)


try:
    from concourse import bacc, mybir, tile  # noqa: F401
except ImportError:
    _ensure_paths()
    from concourse import bacc, mybir, tile  # noqa: F401

from concourse.bass_utils import run_bass_kernel_spmd
from concourse.masks import make_identity

N_CORES = 8
IMGS = 8          # images per core (64 / 8)
C = 256
CB = 2            # channel blocks of 128
H = W = 28
HP = WP = 30      # zero-padded spatial
PIX = H * W       # 784
HALF = PIX // 2   # 392 (one PSUM bank of fp32)
NT = 64 * PIX     # BN count over the GLOBAL batch (N*H*W)
EPS = 1e-5
NPAD = IMGS + 2   # physical padded-activation buffers (recycled)

F32 = mybir.dt.float32
BF16 = mybir.dt.bfloat16
FP8 = mybir.dt.float8e4
AF = mybir.ActivationFunctionType
ALU = mybir.AluOpType
DR = mybir.MatmulPerfMode.DoubleRow

# padded fp8 activation layout: [128, 2 kblocks, 30 rows, 32 cols]
RP = 32           # row pitch (28 cols + pad, %16 bytes)
KP = HP * RP      # per-kblock pitch = 960

_PROGRAM = None


def _build_program():
    nc = bacc.Bacc("TRN2", target_bir_lowering=False, debug=False,
                   num_devices=N_CORES)

    x_in = nc.dram_tensor("x", [IMGS, C, H, W], BF16, kind="ExternalInput").ap()
    w1_in = nc.dram_tensor("w1", [C, C, 3, 3], BF16, kind="ExternalInput").ap()
    w2_in = nc.dram_tensor("w2", [C, C, 3, 3], BF16, kind="ExternalInput").ap()
    g1_in = nc.dram_tensor("gamma1", [C], F32, kind="ExternalInput").ap()
    b1_in = nc.dram_tensor("beta1", [C], F32, kind="ExternalInput").ap()
    g2_in = nc.dram_tensor("gamma2", [C], F32, kind="ExternalInput").ap()
    b2_in = nc.dram_tensor("beta2", [C], F32, kind="ExternalInput").ap()
    out_d = nc.dram_tensor("out", [IMGS, C, H, W], F32, kind="ExternalOutput").ap()

    groups = [list(range(N_CORES))]

    with tile.TileContext(nc) as tc:
        with (
            tc.tile_pool(name="consts", bufs=1) as p_const,
            tc.tile_pool(name="wstage", bufs=3) as p_wstage,
            tc.tile_pool(name="wsign", bufs=2) as p_wsign,
            tc.tile_pool(name="wt", bufs=2 * 9 * 2) as p_wt,
            tc.tile_pool(name="xp", bufs=IMGS * CB) as p_x,
            tc.tile_pool(name="apad", bufs=NPAD) as p_apad,
            tc.tile_pool(name="yz", bufs=IMGS * CB) as p_yz,
            tc.tile_pool(name="sq", bufs=2) as p_sq,
            tc.tile_pool(name="o1", bufs=8) as p_o1,
            tc.tile_pool(name="ps", bufs=8, space="PSUM") as p_ps,
            tc.tile_pool(name="dram", bufs=1, space="DRAM") as p_dram,
        ):

            # bf16 identity: weight transposes run at 1 cycle/row in bf16
            # (vs 2 for f32), halving their PE cost.
            ident = p_const.tile([128, 128], BF16, name="ident")
            make_identity(nc, ident)

            def dma_chunked(out_ap, in_ap, parts, engs):
                """Split a big DMA along the last free dim so the transfer
                spreads across several DMA engines; rotate the issuing
                queue over `engs` so the ~0.6us per-issue cost is
                parallelized across sequencers."""
                n = out_ap.shape[-1]
                step = (n + parts - 1) // parts
                for ci, a in enumerate(range(0, n, step)):
                    b = min(a + step, n)
                    engs[ci % len(engs)].dma_start(
                        out=out_ap[:, a:b], in_=in_ap[:, a:b])

            # gamma/beta as [128, 2] (col = channel block)
            def load_cvec(src, nm):
                t = p_const.tile([128, CB], F32, name=nm)
                nc.sync.dma_start(out=t, in_=src.rearrange("(b p) -> p b", p=128))
                return t

            # gamma/beta loads deferred (cvec_loads) so they don't occupy
            # DMA slots during the critical head window.
            cv = {}

            def cvec_loads():
                g1t = load_cvec(g1_in, "g1t")
                b1t = load_cvec(b1_in, "b1t")
                rg1 = p_const.tile([128, CB], F32, name="rg1")
                nc.vector.reciprocal(rg1, g1t)
                bg1 = p_const.tile([128, CB], F32, name="bg1")
                nc.vector.tensor_mul(bg1, b1t, rg1)
                cv["bg1"] = bg1
                cv["g2t"] = load_cvec(g2_in, "g2t")
                cv["b2t"] = load_cvec(b2_in, "b2t")

            # per-channel stat accumulators, one column per (img, half)
            def stat_tiles(nm):
                return [p_const.tile([128, IMGS * 2], F32, name=f"{nm}{ob}")
                        for ob in range(CB)]

            st1s, st1q = stat_tiles("st1s"), stat_tiles("st1q")
            st2s, st2q = stat_tiles("st2s"), stat_tiles("st2q")

            # ---- padded fp8 activation buffers: border-only memsets ----
            # The binarize writes always cover [1:29, 1:29]; only the halo
            # (row 0, row 29, col 0, cols 29-31) must be zero, and it stays
            # zero when a buffer is recycled between conv1 and conv2 inputs.
            pad = [p_apad.tile([128, CB * KP], FP8, tag="apad", name=f"pad{i}")
                   for i in range(NPAD)]

            def memset_border(t):
                a4 = t.rearrange("p (k r c) -> p k r c", k=CB, r=HP)
                for b in range(CB):
                    nc.gpsimd.memset(a4[:, b, 0, :], 0.0)
                    nc.gpsimd.memset(a4[:, b, 29, :], 0.0)
                    nc.gpsimd.memset(a4[:, b, 1:29, 0:1], 0.0)
                    nc.gpsimd.memset(a4[:, b, 1:29, 29:32], 0.0)

            xsign = [pad[n] for n in range(IMGS)]
            b2a = [pad[(IMGS + n) % NPAD] for n in range(IMGS)]

            # ---- weight staging (DMA) and prep: one big Sign to bf16, then
            # cheap 1-cycle/row PE transposes, then PSUM->fp8 copies ----
            def stage_w(w_in, wi, ob):
                wst = p_wstage.tile([128, C * 9], BF16, tag="wst",
                                    name=f"wst{wi}_{ob}")
                dma_chunked(
                    wst,
                    w_in[ob * 128:(ob + 1) * 128].rearrange(
                        "o i ky kx -> o (i ky kx)"),
                    parts=6,
                    engs=[nc.sync, nc.scalar] if (wi, ob) == (1, 0)
                    else [nc.sync])
                return wst

            def prep_w(wst, wi, ob, wt, on_vector=False):
                """Sign + transpose + PSUM->fp8 copy for one weight block.
                on_vector: run the Sign on the Vector engine as
                is_ge - 0.5 (weights become +-0.5 instead of +-1; the
                eviction scale compensates), keeping the Scalar queue free.
                The PSUM copies alternate Scalar/Vector."""
                wsg = p_wsign.tile([128, C * 9], BF16, tag="wsg",
                                   name=f"wsg{wi}_{ob}")
                if on_vector:
                    nc.vector.tensor_scalar(out=wsg, in0=wst, scalar1=0.0,
                                            scalar2=0.5, op0=ALU.is_ge,
                                            op1=ALU.subtract)
                else:
                    nc.scalar.activation(wsg, wst, AF.Sign)
                w3 = wsg.rearrange("p (i t) -> p i t", t=9)
                for tap in range(9):
                    t = p_wt.tile([128, CB * 128], FP8, tag="wt",
                                  name=f"wt{wi}_{tap}_{ob}")
                    wt[(tap, ob)] = t
                    for kb in range(CB):
                        ps = p_ps.tile([128, 128], BF16, tag="ps",
                                       name=f"pst{wi}_{ob}_{kb}_{tap}")
                        nc.tensor.transpose(
                            ps, w3[:, kb * 128:(kb + 1) * 128, tap], ident)
                        if kb == 0:
                            nc.scalar.activation(
                                t[:, kb * 128:(kb + 1) * 128], ps, AF.Copy)
                        else:
                            nc.vector.tensor_scalar(
                                out=t[:, kb * 128:(kb + 1) * 128], in0=ps,
                                scalar1=1.0, scalar2=None, op0=ALU.mult)

            # ---- x: load raw f32 (kept for the residual), sign -> fp8 ----
            xt = [[None] * CB for _ in range(IMGS)]

            def load_x(n, parts):
                a4 = xsign[n].rearrange("p (k r c) -> p k r c", k=CB, r=HP)
                for b in range(CB):
                    xr = p_x.tile([128, PIX], BF16, tag="xp", name=f"x_{n}_{b}")
                    dma_chunked(
                        xr,
                        x_in[n, b * 128:(b + 1) * 128].rearrange(
                            "c h w -> c (h w)"),
                        parts=parts,
                        engs=[nc.sync, nc.scalar] if n < 2 else [nc.sync])
                    xt[n][b] = xr
                    if n < 2:
                        # Scalar-engine Sign (+-1 acts, evict1 scale 1.0):
                        # keeps the head's Vector queue free for the
                        # weight-prep PSUM copies.
                        nc.scalar.activation(
                            a4[:, b, 1:29, 1:29],
                            xr.rearrange("p (h w) -> p h w", h=H),
                            AF.Sign)
                    else:
                        nc.vector.tensor_scalar(
                            out=a4[:, b, 1:29, 1:29],
                            in0=xr.rearrange("p (h w) -> p h w", h=H),
                            scalar1=0.0, scalar2=0.5,
                            op0=ALU.is_ge, op1=ALU.subtract)

            # head: w1-ob0 staging + the first image pair lead the DMA
            # queues (nothing else competes until they are in flight), then
            # w1-ob1 and the x stream; w2 staging is deferred until after
            # conv1 is emitted so its transfers ride behind the x stream.
            # GpSimd only runs the halo memsets here; the first two pad
            # buffers are zeroed before the img0/img1 binarize needs them.
            wt1, wt2 = {}, {}
            memset_border(pad[0])
            memset_border(pad[1])
            # Warmup collective: absorbs the ~11us ncfw wake + first-mesh-op
            # overhead on stream 0 while conv1 runs, so the BN sync
            # AllReduces behave like warm ops. Emitted after the first two
            # halo memsets: its trigger blocks the GpSimd queue ~7us.
            ccw_i = p_dram.tile([128, 1], F32, name="ccw_i")
            ccw_o = p_dram.tile([128, 1], F32, name="ccw_o")
            zz = p_const.tile([128, 1], F32, name="zz")
            nc.vector.memset(zz, 0.0)
            nc.scalar.dma_start(out=ccw_i, in_=zz)
            nc.gpsimd.collective_compute(
                "AllReduce", ALU.add, replica_groups=groups,
                ins=[ccw_i.opt()], outs=[ccw_o.opt()])
            load_x(0, parts=2)
            ws10 = stage_w(w1_in, 1, 0)
            load_x(1, parts=2)
            prep_w(ws10, 1, 0, wt1)
            ws11 = stage_w(w1_in, 1, 1)
            cvec_loads()
            for n in range(2, IMGS):
                load_x(n, parts=2 if n < 4 else 1)
            for i in range(2, NPAD):
                memset_border(pad[i])

            # ---- conv: 9 DoubleRow matmuls (K=256) per [128, 392] PSUM ----
            def emit_group(wt, act, evict, pair, ob, tiles=None):
                if tiles is None:
                    tiles = [(n, half)
                             for n in (2 * pair, 2 * pair + 1)
                             for half in range(2)]
                pss = {}
                for (n, half) in tiles:
                    pss[(n, half)] = p_ps.tile(
                        [128, HALF], F32, tag="ps",
                        name=f"ps_{ob}_{n}_{half}")
                for tap in range(9):
                    dy, dx = divmod(tap, 3)
                    w3 = wt[(tap, ob)].rearrange(
                        "p (k o) -> p k o", k=CB)
                    for (n, half) in tiles:
                        a4 = act[n].rearrange(
                            "p (k r c) -> p k r c", k=CB, r=HP)
                        rhs = a4[:, :, dy + half * 14: dy + half * 14 + 14,
                                 dx: dx + W]
                        nc.tensor.matmul(pss[(n, half)], w3, rhs,
                                         start=(tap == 0),
                                         stop=(tap == 8),
                                         perf_mode=DR)
                for (n, half) in tiles:
                    evict(n, ob, half, pss[(n, half)])

            # ---- conv1 eviction: copy PSUM->y1 with sum, square w/ sumsq ----
            y1 = [[None] * CB for _ in range(IMGS)]

            def evict1(n, ob, half, ps):
                if y1[n][ob] is None:
                    y1[n][ob] = p_yz.tile([128, PIX], F32, tag="yz",
                                          name=f"y1_{n}_{ob}")
                idx = n * 2 + half
                ysl = y1[n][ob][:, half * HALF:(half + 1) * HALF]
                nc.scalar.activation(ysl, ps, AF.Copy,
                                     scale=1.0 if n < 2 else 2.0,
                                     accum_out=st1s[ob][:, idx:idx + 1])
                sq = p_sq.tile([128, HALF], F32, tag="sq")
                nc.vector.scalar_tensor_tensor(
                    out=sq, in0=ysl, scalar=1.0, in1=ysl,
                    op0=ALU.mult, op1=ALU.mult,
                    accum_out=st1q[ob][:, idx:idx + 1])

            # conv1: image-major (pairs outer) relaxes the x-load deadlines.
            # The first two groups are single-image so the very first
            # matmuls need only img0; w1-ob1 prep slots in between.
            emit_group(wt1, xsign, evict1, 0, 0, tiles=[(0, 0), (0, 1)])
            emit_group(wt1, xsign, evict1, 0, 0, tiles=[(1, 0), (1, 1)])
            prep_w(ws11, 1, 1, wt1)
            emit_group(wt1, xsign, evict1, 0, 1)
            for pair in range(1, IMGS // 2):
                for ob in range(CB):
                    emit_group(wt1, xsign, evict1, pair, ob)

            # w2 staging: issued here so the transfers queue up behind the
            # x stream and land well before the BN1 sync window.
            ws20 = stage_w(w2_in, 2, 0)
            ws21 = stage_w(w2_in, 2, 1)

            # ---- BN1: AllReduce global sums, derive per-channel thresholds ----
            # free-axis reduce as a GpSimd add-tree: the GpSimd queue
            # holds the whole sync path (reduce -> cc-in DMA -> trigger ->
            # readback -> param chain), so busy Vector/Scalar queues can
            # never delay a BN sync.
            rs8 = p_const.tile([128, 8], F32, name="rs8")
            rs4 = p_const.tile([128, 4], F32, name="rs4")
            rs2 = p_const.tile([128, 2], F32, name="rs2")

            def tree_reduce(dst, src_t):
                nc.gpsimd.tensor_tensor(out=rs8, in0=src_t[:, 0:8],
                                        in1=src_t[:, 8:16], op=ALU.add)
                nc.gpsimd.tensor_tensor(out=rs4, in0=rs8[:, 0:4],
                                        in1=rs8[:, 4:8], op=ALU.add)
                nc.gpsimd.tensor_tensor(out=rs2, in0=rs4[:, 0:2],
                                        in1=rs4[:, 2:4], op=ALU.add)
                nc.gpsimd.tensor_tensor(out=dst, in0=rs2[:, 0:1],
                                        in1=rs2[:, 1:2], op=ALU.add)

            pk1 = p_const.tile([128, 2 * CB], F32, name="pk1")
            for ob in range(CB):
                tree_reduce(pk1[:, 2 * ob:2 * ob + 1], st1s[ob])
                tree_reduce(pk1[:, 2 * ob + 1:2 * ob + 2], st1q[ob])
            cc1i = p_dram.tile([128, 2 * CB], F32, name="cc1i")
            cc1o = p_dram.tile([128, 2 * CB], F32, name="cc1o")
            nc.sync.dma_start(out=cc1i, in_=pk1)
            nc.gpsimd.collective_compute(
                "AllReduce", ALU.add, replica_groups=groups,
                ins=[cc1i.opt()], outs=[cc1o.opt()])
            red1 = p_const.tile([128, 2 * CB], F32, name="red1")
            nc.sync.dma_start(out=red1, in_=cc1o)
            r3 = red1.rearrange("p (b k) -> p b k", k=2)

            # w2 prep fills the sync-BN wait on the PE
            prep_w(ws20, 2, 0, wt2, on_vector=True)
            prep_w(ws21, 2, 1, wt2, on_vector=True)

            # threshold chain on GpSimd (idle here) to keep it off the
            # busier Vector queue; only the Sqrt hops to Scalar.
            m1 = p_const.tile([128, CB], F32, name="m1")
            nc.gpsimd.tensor_scalar(out=m1, in0=r3[:, :, 0], scalar1=1.0 / NT,
                                    scalar2=None, op0=ALU.mult)
            mm1 = p_const.tile([128, CB], F32, name="mm1")
            nc.gpsimd.tensor_mul(mm1, m1, m1)
            v1e = p_const.tile([128, CB], F32, name="v1e")
            nc.gpsimd.tensor_scalar(out=v1e, in0=r3[:, :, 1],
                                    scalar1=1.0 / NT, scalar2=EPS,
                                    op0=ALU.mult, op1=ALU.add)
            v1 = p_const.tile([128, CB], F32, name="v1")
            nc.gpsimd.tensor_sub(v1, v1e, mm1)
            sd1 = p_const.tile([128, CB], F32, name="sd1")
            nc.scalar.activation(sd1, v1, AF.Sqrt)
            tb1 = p_const.tile([128, CB], F32, name="tb1")
            nc.gpsimd.tensor_mul(tb1, cv["bg1"], sd1)
            thr1 = p_const.tile([128, CB], F32, name="thr1")
            nc.gpsimd.tensor_sub(thr1, m1, tb1)
            # negated threshold: bias for the Scalar-engine Sign binarize
            thn1 = p_const.tile([128, CB], F32, name="thn1")
            nc.gpsimd.tensor_sub(thn1, tb1, m1)

            # ---- binarize(BN1(y1)): imgs 0-1 on Scalar as Sign(y - thr)
            # (values +-1, conv2 evict scale 1.0) so conv2 starts sooner;
            # the rest on Vector as is_ge - 0.5 (values +-0.5, scale 2). ----
            for n in range(IMGS):
                a4 = b2a[n].rearrange("p (k r c) -> p k r c", k=CB, r=HP)
                for b in range(CB):
                    if n < 2:
                        nc.scalar.activation(
                            a4[:, b, 1:29, 1:29],
                            y1[n][b].rearrange("p (h w) -> p h w", h=H),
                            AF.Sign, bias=thn1[:, b:b + 1])
                    else:
                        nc.vector.tensor_scalar(
                            out=a4[:, b, 1:29, 1:29],
                            in0=y1[n][b].rearrange("p (h w) -> p h w", h=H),
                            scalar1=thr1[:, b:b + 1], scalar2=0.5,
                            op0=ALU.is_ge, op1=ALU.subtract)

            # ---- conv2 eviction: z = s*psum + x (fused sum), square ----
            z = [[None] * CB for _ in range(IMGS)]

            def evict2(n, ob, half, ps):
                if z[n][ob] is None:
                    z[n][ob] = p_yz.tile([128, PIX], F32, tag="yz",
                                         name=f"z_{n}_{ob}")
                idx = n * 2 + half
                zsl = z[n][ob][:, half * HALF:(half + 1) * HALF]
                nc.vector.scalar_tensor_tensor(
                    out=zsl, in0=ps, scalar=2.0 if n < 2 else 4.0,
                    in1=xt[n][ob][:, half * HALF:(half + 1) * HALF],
                    op0=ALU.mult, op1=ALU.add,
                    accum_out=st2s[ob][:, idx:idx + 1])
                sq = p_sq.tile([128, HALF], F32, tag="sq")
                if half == 0 and n < 6:
                    nc.scalar.activation(sq, zsl, AF.Square,
                                         accum_out=st2q[ob][:, idx:idx + 1])
                else:
                    nc.vector.scalar_tensor_tensor(
                        out=sq, in0=zsl, scalar=1.0, in1=zsl,
                        op0=ALU.mult, op1=ALU.mult,
                        accum_out=st2q[ob][:, idx:idx + 1])

            # ---- BN2 per output block: reduce, AllReduce, affine params.
            # The post-AR chain runs on GpSimd (idle then) + one Scalar
            # Rsqrt so the busy Vector/Sync queues never gate fs/fb. ----
            def bn2_ar(ob):
                pk = p_const.tile([128, 2], F32, name=f"pk2_{ob}")
                tree_reduce(pk[:, 0:1], st2s[ob])
                tree_reduce(pk[:, 1:2], st2q[ob])
                cci = p_dram.tile([128, 2], F32, name=f"cc2i_{ob}")
                cco = p_dram.tile([128, 2], F32, name=f"cc2o_{ob}")
                nc.sync.dma_start(out=cci, in_=pk)
                nc.gpsimd.collective_compute(
                    "AllReduce", ALU.add, replica_groups=groups,
                    ins=[cci.opt()], outs=[cco.opt()])
                red = p_const.tile([128, 2], F32, name=f"red2_{ob}")
                nc.sync.dma_start(out=red, in_=cco)
                return red

            def bn2_chain(red, ob):
                # post-AR chain on Vector, emitted only after every conv2
                # eviction so a strict in-order schedule cannot stall them
                m2 = p_const.tile([128, 1], F32, name=f"m2_{ob}")
                nc.vector.tensor_scalar(out=m2, in0=red[:, 0:1],
                                        scalar1=1.0 / NT, scalar2=None,
                                        op0=ALU.mult)
                mm2 = p_const.tile([128, 1], F32, name=f"mm2_{ob}")
                nc.vector.tensor_scalar(out=mm2, in0=red[:, 0:1],
                                        scalar1=red[:, 0:1],
                                        scalar2=1.0 / (NT * NT),
                                        op0=ALU.mult, op1=ALU.mult)
                v2f = p_const.tile([128, 1], F32, name=f"v2f_{ob}")
                nc.vector.tensor_scalar(out=v2f, in0=red[:, 1:2],
                                        scalar1=1.0 / NT, scalar2=EPS,
                                        op0=ALU.mult, op1=ALU.add)
                v2 = p_const.tile([128, 1], F32, name=f"v2_{ob}")
                nc.vector.tensor_sub(v2, v2f, mm2)
                rc2 = p_const.tile([128, 1], F32, name=f"rc2_{ob}")
                nc.vector.reciprocal(rc2, v2)
                rstd = p_const.tile([128, 1], F32, name=f"rstd_{ob}")
                nc.scalar.activation(rstd, rc2, AF.Sqrt)
                fs = p_const.tile([128, 1], F32, name=f"fs_{ob}")
                nc.vector.tensor_mul(fs, cv["g2t"][:, ob:ob + 1], rstd)
                msc = p_const.tile([128, 1], F32, name=f"msc_{ob}")
                nc.vector.tensor_mul(msc, m2, fs)
                fb = p_const.tile([128, 1], F32, name=f"fb_{ob}")
                nc.vector.tensor_sub(fb, cv["b2t"][:, ob:ob + 1], msc)
                return fs, fb

            # ---- final: clip(z * fscale + fbias) -> DRAM ----
            def out_tile(n, ob, fs, fb, aff_eng, clamp_eng, dma_engs):
                o1 = p_o1.tile([128, PIX], F32, tag="o1")
                if aff_eng is nc.scalar:
                    nc.scalar.activation(o1, z[n][ob], AF.Identity,
                                         bias=fb, scale=fs)
                else:
                    aff_eng.tensor_scalar(
                        out=o1, in0=z[n][ob], scalar1=fs, scalar2=fb,
                        op0=ALU.mult, op1=ALU.add)
                clamp_eng.tensor_scalar(out=o1, in0=o1, scalar1=-1.0,
                                        scalar2=1.0, op0=ALU.max, op1=ALU.min)
                dma_chunked(
                    out_d[n, ob * 128:(ob + 1) * 128].rearrange(
                        "c h w -> c (h w)"),
                    o1, parts=1, engs=dma_engs)

            # conv2-ob0, then its BN sync; the ob0 output phase rides on
            # GpSimd (affine) + Vector (clamp; GpSimd MAX/MIN is ~10x slow)
            # + the sync DMA queue underneath conv2-ob1's PE time.
            for pair in range(IMGS // 2):
                emit_group(wt2, b2a, evict2, pair, 0)
            red2a = bn2_ar(0)
            for pair in range(IMGS // 2):
                emit_group(wt2, b2a, evict2, pair, 1)
            red2b = bn2_ar(1)

            fs0, fb0 = bn2_chain(red2a, 0)
            for n in range(IMGS):
                aff = nc.scalar if n % 2 == 0 else nc.vector
                out_tile(n, 0, fs0, fb0, aff, nc.vector, [nc.sync])

            fs1, fb1 = bn2_chain(red2b, 1)
            for n in range(IMGS):
                aff = nc.scalar if n % 2 == 0 else nc.gpsimd
                dmae = [[nc.sync], [nc.scalar]][n % 2]
                out_tile(n, 1, fs1, fb1, aff, nc.vector, dmae)

    nc.compile()
    return nc


def _get_program():
    global _PROGRAM
    if _PROGRAM is None:
        _PROGRAM = _build_program()
    return _PROGRAM


def run_sharded(inputs, **spmd_kwargs):
    """Shard inputs across 8 cores, run, and gather. Returns (out, results)."""
    nc = _get_program()
    bf16 = mybir.dt.np(mybir.dt.bfloat16)
    x = np.ascontiguousarray(np.asarray(inputs["x"]).astype(bf16))
    base = {}
    for k in ("w1", "w2"):
        base[k] = np.ascontiguousarray(np.asarray(inputs[k]).astype(bf16))
    for k in ("gamma1", "beta1", "gamma2", "beta2"):
        base[k] = np.ascontiguousarray(
            np.asarray(inputs[k], dtype=np.float32))
    shards = np.split(x, N_CORES, axis=0)
    in_maps = [{"x": shards[i], **base} for i in range(N_CORES)]
    res = run_bass_kernel_spmd(nc, in_maps, core_ids=list(range(N_CORES)),
                               **spmd_kwargs)
    out = np.concatenate([res.results[i]["out"] for i in range(N_CORES)],
                         axis=0).astype(np.float32)
    return out, res


def kernel(**inputs):
    out, _ = run_sharded(inputs)
    return out
